# revision 58
# baseline (speedup 1.0000x reference)
"""CQT extractor kernel for Trainium2 (8 NeuronCores, data-parallel over batch).

Pipeline per core (2 audio rows): STFT-as-matmul truncated to the lowest 512
of 1025 rfft bins (CQT weights above ~5.5 kHz are < 2e-3 and contribute ~1e-4
relative error), hybrid precision:
  - freq block 0 (fade-critical narrow CQT bins): fp16, unfolded -- both
    staged streams feed the PE directly (cos fold symmetry reuses one table;
    a negated-sin table handles the rev stream for im).
  - freq blocks 1..3 (wide, noise-averaging bins): fp8 e4m3 with Hermitian
    folding and DoubleRow matmuls (two 128-chunk k-tiles per pass).
Magnitude via ACT Square/Sqrt with the |.|^2 sum on gpsimd, CQT projection
matmul in fp16, log10 via Ln.

The host pre-transposes the reflect-padded audio into per-tile SBUF-layout
blocks (pure data movement), so the device does no transposes or PSUM->SBUF
staging copies; the in-order engine queues carry disjoint streams (DVE: fp8
folds, Pool: magnitude sums, scalar: activations) with DMA prefetched two
tiles ahead.
"""

import math
from contextlib import ExitStack

import numpy as np

import concourse.tile as tile
from concourse import bacc, mybir
from concourse.bass_utils import run_bass_kernel_spmd

# ---- problem constants (hardcoded per contest rules) ----
B = 16
L = 1310720
SR = 22050
HOP = 512
NFFT = 2048
NBINS = 84
BPO = 12
FMIN = 27.5

NF = 1 + L // HOP            # 2561 frames
PAD = NFFT // 2              # 1024
LP = L + 2 * PAD             # 1312768 reflect-padded length

NCORES = 8
ROWS_PER_CORE = B // NCORES  # 2

NFREQ = 512                  # truncated rfft bins (of 1025)
NI = NFREQ // 128            # 4 freq blocks

# frame tiling: 6 uniform tiles of 428 frames; frames past NF-1 are computed
# on zero padding and never written out
T_SIZES = [428] * 6
T_STARTS = [428 * i for i in range(6)]
T_ALLOC = 428

NGRP = 14                    # 128-chunk transpose groups per frame tile
WCH = NGRP * 128             # 1792 chunks staged per frame tile
Q = WCH // 4                 # 448 per-phase chunk columns
NCH_PAD = 4 * T_STARTS[-1] + WCH + 1   # chunks incl. zero pad (+1 for +1 shift)
PADLEN = 128 * NCH_PAD

F32 = mybir.dt.float32
F16 = mybir.dt.float16
F8 = mybir.dt.float8e4
LOG10E = 1.0 / math.log(10.0)


def _host_tables():
    """Folded DFT matrices (512 bins) and CQT weights, float64 -> float16."""
    j = np.arange(1024)
    n = (j + 1).astype(np.float64)          # contraction index j <-> sample n=j+1
    win = 0.5 * (1.0 - np.cos(2.0 * np.pi * n / NFFT))
    ang = 2.0 * np.pi * np.outer(n, np.arange(NFREQ, dtype=np.float64)) / NFFT
    wc = win[:, None] * np.cos(ang)
    ws = win[:, None] * np.sin(ang)
    wc[1023] *= 0.5           # n=1024 term is double-counted by the fold
    ws[1023] = 0.0
    sf = np.fft.rfftfreq(NFFT, 1.0 / SR)[:NFREQ]
    cf = FMIN * 2.0 ** (np.arange(NBINS, dtype=np.float64) / BPO)
    wq = np.exp(-np.abs(sf[:, None] - cf[None, :]) / (0.1 * cf[None, :]))
    return (
        np.ascontiguousarray(wc, dtype=np.float16),
        np.ascontiguousarray(ws, dtype=np.float16),
        np.ascontiguousarray(wq, dtype=np.float16),
    )


def _build_program():
    nc = bacc.Bacc("TRN2", target_bir_lowering=False, debug=False,
                   num_devices=NCORES)
    # host-pretransposed staged audio, per frame tile:
    # xq[r, it, p, ph, q] = xpad[128*(4*T_STARTS[it] + 4q + ph) + p + 1]
    # zq[r, it, p, b,  q] = xpad[128*(4*T_STARTS[it] + 4q + 3 - b) + 127 - p]
    xq = nc.dram_tensor("xq", [ROWS_PER_CORE, 6, 128, 4, Q], F16,
                        kind="ExternalInput").ap()
    zq = nc.dram_tensor("zq", [ROWS_PER_CORE, 6, 128, 4, Q], F16,
                        kind="ExternalInput").ap()
    # fp16 tables for freq block 0 (fade-critical), incl. negated sin for the
    # unfolded rev-stream im accumulation; fp8 tables for blocks 1..3
    wc = nc.dram_tensor("wc", [8, 128, 128], F16, kind="ExternalInput").ap()
    ws = nc.dram_tensor("ws", [8, 128, 128], F16, kind="ExternalInput").ap()
    wsn = nc.dram_tensor("wsn", [8, 128, 128], F16, kind="ExternalInput").ap()
    wc8 = nc.dram_tensor("wc8", [NI - 1, 8, 128, 128], F8,
                         kind="ExternalInput").ap()
    ws8 = nc.dram_tensor("ws8", [NI - 1, 8, 128, 128], F8,
                         kind="ExternalInput").ap()
    wq = nc.dram_tensor("wq", [NI, 128, NBINS], F16, kind="ExternalInput").ap()
    out = nc.dram_tensor("out", [ROWS_PER_CORE, NBINS, NF], F32,
                         kind="ExternalOutput").ap()

    with tile.TileContext(nc) as tc:
        with ExitStack() as ctx:
            _emit(ctx, tc, xq, zq, wc, ws, wsn, wc8, ws8, wq, out)
    nc.compile()
    return nc


def _emit(ctx, tc, xq, zq, wc, ws, wsn, wc8, ws8, wq, out):
    nc = tc.nc
    SQ = mybir.ActivationFunctionType.Square
    SQRT = mybir.ActivationFunctionType.Sqrt
    LN = mybir.ActivationFunctionType.Ln
    DR = mybir.MatmulPerfMode.DoubleRow

    consts = ctx.enter_context(tc.tile_pool(name="consts", bufs=1))
    stage = ctx.enter_context(tc.tile_pool(name="stage", bufs=3))
    eo = ctx.enter_context(tc.tile_pool(name="eo", bufs=3))
    magp = ctx.enter_context(tc.tile_pool(name="magp", bufs=2))
    sqp = ctx.enter_context(tc.tile_pool(name="sqp", bufs=2))
    outp = ctx.enter_context(tc.tile_pool(name="outp", bufs=2))
    ps_mm = ctx.enter_context(tc.tile_pool(name="ps_mm", bufs=4, space="PSUM"))
    ps_cq = ctx.enter_context(tc.tile_pool(name="ps_cq", bufs=2, space="PSUM"))

    # constants
    wc_sb = consts.tile([128, 8, 128], F16, tag="wc_sb")
    ws_sb = consts.tile([128, 8, 128], F16, tag="ws_sb")
    wsn_sb = consts.tile([128, 8, 128], F16, tag="wsn_sb")
    wc8_sb = [consts.tile([128, 8, 128], F8, tag=f"wc8_{i}",
                          name=f"wc8sb{i}") for i in range(1, NI)]
    ws8_sb = [consts.tile([128, 8, 128], F8, tag=f"ws8_{i}",
                          name=f"ws8sb{i}") for i in range(1, NI)]
    wq_sb = consts.tile([128, NI, NBINS], F16, tag="wq_sb")
    lnbias = consts.tile([128, 1], F32, tag="lnbias")
    nc.gpsimd.memset(lnbias[:], 1e-10)

    def emit_weight_loads():
        # after tile-0 audio DMAs so the critical path isn't queued behind
        # the tables; block-0 (first-needed) tables first
        nc.gpsimd.dma_start(wc_sb[:], wc.rearrange("a p f -> p a f"))
        nc.scalar.dma_start(ws_sb[:], ws.rearrange("a p f -> p a f"))
        nc.gpsimd.dma_start(wsn_sb[:], wsn.rearrange("a p f -> p a f"))
        for i in range(1, NI):
            nc.gpsimd.dma_start(wc8_sb[i - 1][:],
                                wc8[i - 1].rearrange("a p f -> p a f"))
            nc.scalar.dma_start(ws8_sb[i - 1][:],
                                ws8[i - 1].rearrange("a p f -> p a f"))
        nc.scalar.dma_start(wq_sb[:], wq.rearrange("i p k -> p i k"))

    def emit_stage_dma(r, it, rev_q=None, split=False):
        """DMA of host-pretransposed chunks.

        split=True (tile 0 only): per-phase slices, so the first i0 matmul
        (which reads only phase a%4=0) starts after ~1/4 of the transfer.
        """
        dts = stage.tile([128, 4, Q], F16, tag="dts")
        rev = stage.tile([128, 4, Q], F16, tag="rev")
        if split:
            for ph in range(4):
                nc.sync.dma_start(dts[:, ph], xq[r, it, :, ph])
                (rev_q or nc.gpsimd).dma_start(rev[:, ph], zq[r, it, :, ph])
        else:
            nc.sync.dma_start(dts[:], xq[r, it])
            (rev_q or nc.gpsimd).dma_start(rev[:], zq[r, it])
        return dts, rev

    def emit_fold(it, dts, rev):
        """fp8 folded operands for freq blocks 1..3 (block 0 reads dts/rev
        directly, unfolded, in fp16).

        dts[p,ph,qq] = chunk(4qq+ph) sample p (shifted stream);
        rev[p,b,qq] = z-chunk(4qq+3-b), so partner of e8[:,a] (chunk
        4t+15-a) sits at rev[:, a%4, 3-(a//4)+t] -- two wide adds cover
        a=0..3 and a=4..7 with unit-stride operands.
        """
        T = T_SIZES[it]
        e8 = eo.tile([128, 8, T_ALLOC], F8, tag="e8")
        o8 = eo.tile([128, 8, T_ALLOC], F8, tag="o8")
        nc.vector.tensor_add(e8[:, 0:4, :T], dts[:, :, 0:T], rev[:, :, 3:3 + T])
        nc.vector.tensor_add(e8[:, 4:8, :T], dts[:, :, 1:1 + T], rev[:, :, 2:2 + T])
        nc.vector.tensor_sub(o8[:, 0:4, :T], dts[:, :, 0:T], rev[:, :, 3:3 + T])
        nc.vector.tensor_sub(o8[:, 4:8, :T], dts[:, :, 1:1 + T], rev[:, :, 2:2 + T])
        return dts, rev, e8, o8

    def emit_dft(r, it, dts, rev, e8, o8, split_tail=False):
        """DFT matmuls + magnitude for one frame tile (512 bins).

        Block 0 (fade-critical) is fp16 and unfolded: re = wc.x1 + wc.x2,
        im = ws.x1 - ws.x2 over the raw dts/rev streams (cos fold symmetry
        makes the rev-stream cos weights identical). Blocks 1..3 use fp8
        folded operands with DoubleRow (two 128-chunks per pass).
        """
        T = T_SIZES[it]
        H = T // 2
        mag = magp.tile([128, NI, T_ALLOC], F16, tag="mag")
        sq = sqp.tile([128, NI, T_ALLOC], F32, tag="sq")
        sq2 = sqp.tile([128, NI, T_ALLOC], F32, tag="sq2")
        d_ap = [dts[:, a % 4, a // 4: a // 4 + T] for a in range(8)]
        r_ap = [rev[:, a % 4, 3 - a // 4: 3 - a // 4 + T] for a in range(8)]
        for i in range(NI):
            ps_re = ps_mm.tile([128, T_ALLOC], F32, tag="mm")
            ps_im = ps_mm.tile([128, T_ALLOC], F32, tag="mm")
            if i == 0:
                for a in range(8):
                    nc.tensor.matmul(ps_re[:, :T], wc_sb[:, a], d_ap[a],
                                     start=(a == 0), stop=False)
                for a in range(8):
                    nc.tensor.matmul(ps_re[:, :T], wc_sb[:, a], r_ap[a],
                                     start=False, stop=(a == 7))
                for a in range(8):
                    nc.tensor.matmul(ps_im[:, :T], ws_sb[:, a], d_ap[a],
                                     start=(a == 0), stop=False)
                for a in range(8):
                    nc.tensor.matmul(ps_im[:, :T], wsn_sb[:, a], r_ap[a],
                                     start=False, stop=(a == 7))
            else:
                # full-T DoubleRow: moving rows = 2T = 856, but the pair
                # consumption halves the effective pass count so the PSUM
                # output span (T <= 512, one bank) is the binding limit
                for ps, wt, op in ((ps_re, wc8_sb[i - 1], e8),
                                   (ps_im, ws8_sb[i - 1], o8)):
                    for a in range(0, 8, 2):
                        nc.tensor.matmul(
                            ps[:, :T], wt[:, a:a + 2, :],
                            op[:, a:a + 2, :T],
                            start=(a == 0), stop=(a == 6),
                            perf_mode=DR,
                        )
            nc.scalar.activation(sq[:, i, :T], ps_re[:, :T], SQ)
            nc.scalar.activation(sq2[:, i, :T], ps_im[:, :T], SQ)
        # one wide |.|^2 sum + sqrt for the whole tile; the add lives on
        # gpsimd so the DVE queue carries nothing but the fold stream
        if not split_tail:
            # one wide |.|^2 sum + sqrt for the whole tile; the add lives on
            # gpsimd so the DVE queue carries nothing but the fold stream
            nc.gpsimd.tensor_add(sq[:, :, :T], sq[:, :, :T], sq2[:, :, :T])
            nc.scalar.activation(mag[:, :, :T], sq[:, :, :T], SQRT)
        else:
            # last tile: per-block sum/sqrt on the now-idle DVE so the final
            # cqt matmuls can fire incrementally as each mag block lands
            for i in range(NI):
                nc.vector.tensor_add(sq[:, i, :T], sq[:, i, :T], sq2[:, i, :T])
                nc.scalar.activation(mag[:, i, :T], sq[:, i, :T], SQRT)
        return mag

    def emit_cqt(r, it, mag, c0=0, c1=None, tail=False):
        """CQT projection, log10, store (over frame columns [c0, c1))."""
        T = T_SIZES[it]
        f0 = T_STARTS[it]
        if c1 is None:
            c1 = T
        V = min(c1, NF - f0)         # valid (non-garbage) frames
        ps_c = ps_cq.tile([NBINS, T_ALLOC], F32, tag="ps_c")
        for i in range(NI):
            nc.tensor.matmul(
                ps_c[:, c0:c1], wq_sb[:, i, :], mag[:, i, c0:c1],
                start=(i == 0), stop=(i == NI - 1),
            )
        outt = outp.tile([NBINS, T_ALLOC], F32, tag="outt")
        nc.scalar.activation(outt[:, c0:V], ps_c[:, c0:V], LN,
                             bias=lnbias[:NBINS])
        if tail:
            # epilogue: DVE is idle and this avoids Ln<->Copy act-table thrash
            nc.vector.tensor_scalar_mul(outt[:, c0:V], outt[:, c0:V], LOG10E)
        else:
            nc.scalar.mul(outt[:, c0:V], outt[:, c0:V], LOG10E)
        nc.sync.dma_start(out[r, :, f0 + c0: f0 + V], outt[:, c0:V])

    # software pipeline: DMA prefetches two tiles ahead, folds run just-in-
    # time after the current tile's DFT so the in-order DVE queue never
    # blocks the magnitude chain, and cqt(k-1) is emitted AFTER dft(k) so
    # the PE never waits on the k-1 magnitude chain.
    # Slot k: [dma k+2][dft k][cqt k-1][fold k+1]
    tiles = [(r, it) for r in range(ROWS_PER_CORE) for it in range(6)]
    # tile-0 rev rides the scalar queue so wc (needed by the very first
    # matmul) leads the gpsimd queue instead of sitting behind 459KB
    dmas = {0: emit_stage_dma(*tiles[0], rev_q=nc.scalar, split=True)}
    emit_weight_loads()
    dmas[1] = emit_stage_dma(*tiles[1])
    folded = {0: emit_fold(tiles[0][1], *dmas.pop(0))}
    pending = None          # (r, it, mag) awaiting cqt
    last = len(tiles) - 1
    for k, (r, it) in enumerate(tiles):
        if k + 2 < len(tiles):
            dmas[k + 2] = emit_stage_dma(*tiles[k + 2])
        mag = emit_dft(r, it, *folded.pop(k), split_tail=(k == last))
        if pending is not None:
            emit_cqt(*pending)
        pending = (r, it, mag)
        if k + 1 < len(tiles):
            folded[k + 1] = emit_fold(tiles[k + 1][1], *dmas.pop(k + 1))
    emit_cqt(*pending)


_PROGRAM_CACHE = {}


def _get_program():
    if "nc" not in _PROGRAM_CACHE:
        _PROGRAM_CACHE["nc"] = _build_program()
    return _PROGRAM_CACHE["nc"]


def kernel(audio):
    audio = np.asarray(audio, dtype=np.float32)
    assert audio.shape == (B, L), audio.shape

    # host data movement: reflect pad + fp16, then per-tile pretransposed
    # blocks in the exact SBUF layout the device consumes (pure gathers).
    xpad = np.zeros((B, PADLEN), dtype=np.float16)
    xpad[:, :LP] = np.pad(audio, ((0, 0), (PAD, PAD)), mode="reflect")

    xqh = np.empty((B, 6, 128, 4, Q), dtype=np.float16)
    zqh = np.empty((B, 6, 128, 4, Q), dtype=np.float16)
    for it in range(6):
        base = 128 * 4 * T_STARTS[it]
        # xq[p, ph, q] = xpad[base + 128*(4q+ph) + p + 1]
        seg = xpad[:, base + 1: base + 1 + WCH * 128].reshape(B, Q, 4, 128)
        xqh[:, it] = seg.transpose(0, 3, 2, 1)
        # zq[p, b, q] = xpad[base + 128*(4q+3-b) + 127 - p]
        zseg = xpad[:, base: base + WCH * 128].reshape(B, Q, 4, 128)
        zqh[:, it] = zseg[:, :, ::-1, ::-1].transpose(0, 3, 2, 1)

    import ml_dtypes
    wc, ws, wq = _host_tables()
    # block 0 fp16 [a, p, f]; blocks 1..3 fp8 [i-1, a, p, f]
    wc0 = np.ascontiguousarray(wc[:, :128].reshape(8, 128, 128))
    ws0 = np.ascontiguousarray(ws[:, :128].reshape(8, 128, 128))
    wsn0 = np.ascontiguousarray(-ws0)
    e4m3 = ml_dtypes.float8_e4m3fn
    wc8 = np.ascontiguousarray(wc[:, 128:].reshape(8, 128, NI - 1, 128)
                               .transpose(2, 0, 1, 3)).astype(e4m3)
    ws8 = np.ascontiguousarray(ws[:, 128:].reshape(8, 128, NI - 1, 128)
                               .transpose(2, 0, 1, 3)).astype(e4m3)
    wq = np.ascontiguousarray(wq.reshape(NI, 128, NBINS))
    nc = _get_program()

    in_maps = []
    for c in range(NCORES):
        rows = slice(ROWS_PER_CORE * c, ROWS_PER_CORE * (c + 1))
        in_maps.append({
            "xq": np.ascontiguousarray(xqh[rows]),
            "zq": np.ascontiguousarray(zqh[rows]),
            "wc": wc0, "ws": ws0, "wsn": wsn0,
            "wc8": wc8, "ws8": ws8, "wq": wq,
        })

    res = run_bass_kernel_spmd(nc, in_maps, core_ids=list(range(NCORES)))
    out = np.concatenate([res.results[c]["out"] for c in range(NCORES)], axis=0)
    return np.ascontiguousarray(out, dtype=np.float32)


# revision 60
# speedup vs baseline: 1.0042x; 1.0042x over previous
"""CQT extractor kernel for Trainium2 (8 NeuronCores, data-parallel over batch).

Pipeline per core (2 audio rows): STFT-as-matmul truncated to the lowest 512
of 1025 rfft bins (CQT weights above ~5.5 kHz are < 2e-3 and contribute ~1e-4
relative error), hybrid precision:
  - freq block 0 (fade-critical narrow CQT bins): fp16, unfolded -- both
    staged streams feed the PE directly (cos fold symmetry reuses one table;
    a negated-sin table handles the rev stream for im).
  - freq blocks 1..3 (wide, noise-averaging bins): fp8 e4m3 with Hermitian
    folding and DoubleRow matmuls (two 128-chunk k-tiles per pass).
Magnitude via ACT Square/Sqrt with the |.|^2 sum on gpsimd, CQT projection
matmul in fp16, log10 via Ln.

The host pre-transposes the reflect-padded audio into per-tile SBUF-layout
blocks (pure data movement), so the device does no transposes or PSUM->SBUF
staging copies; the in-order engine queues carry disjoint streams (DVE: fp8
folds, Pool: magnitude sums, scalar: activations) with DMA prefetched two
tiles ahead.
"""

import math
from contextlib import ExitStack

import numpy as np

import concourse.tile as tile
from concourse import bacc, mybir
from concourse.bass_utils import run_bass_kernel_spmd

# ---- problem constants (hardcoded per contest rules) ----
B = 16
L = 1310720
SR = 22050
HOP = 512
NFFT = 2048
NBINS = 84
BPO = 12
FMIN = 27.5

NF = 1 + L // HOP            # 2561 frames
PAD = NFFT // 2              # 1024
LP = L + 2 * PAD             # 1312768 reflect-padded length

NCORES = 8
ROWS_PER_CORE = B // NCORES  # 2

NFREQ = 512                  # truncated rfft bins (of 1025)
NI = NFREQ // 128            # 4 freq blocks

# frame tiling: 6 uniform tiles of 428 frames; frames past NF-1 are computed
# on zero padding and never written out
T_SIZES = [428] * 6
T_STARTS = [428 * i for i in range(6)]
T_ALLOC = 428

NGRP = 14                    # 128-chunk transpose groups per frame tile
WCH = NGRP * 128             # 1792 chunks staged per frame tile
Q = WCH // 4                 # 448 per-phase chunk columns
NCH_PAD = 4 * T_STARTS[-1] + WCH + 1   # chunks incl. zero pad (+1 for +1 shift)
PADLEN = 128 * NCH_PAD

F32 = mybir.dt.float32
F16 = mybir.dt.float16
F8 = mybir.dt.float8e4
LOG10E = 1.0 / math.log(10.0)


def _host_tables():
    """Folded DFT matrices (512 bins) and CQT weights, float64 -> float16."""
    j = np.arange(1024)
    n = (j + 1).astype(np.float64)          # contraction index j <-> sample n=j+1
    win = 0.5 * (1.0 - np.cos(2.0 * np.pi * n / NFFT))
    ang = 2.0 * np.pi * np.outer(n, np.arange(NFREQ, dtype=np.float64)) / NFFT
    wc = win[:, None] * np.cos(ang)
    ws = win[:, None] * np.sin(ang)
    wc[1023] *= 0.5           # n=1024 term is double-counted by the fold
    ws[1023] = 0.0
    sf = np.fft.rfftfreq(NFFT, 1.0 / SR)[:NFREQ]
    cf = FMIN * 2.0 ** (np.arange(NBINS, dtype=np.float64) / BPO)
    wq = np.exp(-np.abs(sf[:, None] - cf[None, :]) / (0.1 * cf[None, :]))
    return (
        np.ascontiguousarray(wc, dtype=np.float16),
        np.ascontiguousarray(ws, dtype=np.float16),
        np.ascontiguousarray(wq, dtype=np.float16),
    )


def _build_program():
    nc = bacc.Bacc("TRN2", target_bir_lowering=False, debug=False,
                   num_devices=NCORES)
    # host-pretransposed staged audio, per frame tile:
    # xq[r, it, p, ph, q] = xpad[128*(4*T_STARTS[it] + 4q + ph) + p + 1]
    # zq[r, it, p, b,  q] = xpad[128*(4*T_STARTS[it] + 4q + 3 - b) + 127 - p]
    xq = nc.dram_tensor("xq", [ROWS_PER_CORE, 6, 128, 4, Q], F16,
                        kind="ExternalInput").ap()
    zq = nc.dram_tensor("zq", [ROWS_PER_CORE, 6, 128, 4, Q], F16,
                        kind="ExternalInput").ap()
    # fp16 tables for freq block 0 (fade-critical), incl. negated sin for the
    # unfolded rev-stream im accumulation; fp8 tables for blocks 1..3
    wc = nc.dram_tensor("wc", [8, 128, 128], F16, kind="ExternalInput").ap()
    ws = nc.dram_tensor("ws", [8, 128, 128], F16, kind="ExternalInput").ap()
    wsn = nc.dram_tensor("wsn", [8, 128, 128], F16, kind="ExternalInput").ap()
    wc8 = nc.dram_tensor("wc8", [NI - 1, 8, 128, 128], F8,
                         kind="ExternalInput").ap()
    ws8 = nc.dram_tensor("ws8", [NI - 1, 8, 128, 128], F8,
                         kind="ExternalInput").ap()
    wq = nc.dram_tensor("wq", [NI, 128, NBINS], F16, kind="ExternalInput").ap()
    out = nc.dram_tensor("out", [ROWS_PER_CORE, NBINS, NF], F32,
                         kind="ExternalOutput").ap()

    with tile.TileContext(nc) as tc:
        with ExitStack() as ctx:
            _emit(ctx, tc, xq, zq, wc, ws, wsn, wc8, ws8, wq, out)
    nc.compile()
    return nc


def _emit(ctx, tc, xq, zq, wc, ws, wsn, wc8, ws8, wq, out):
    nc = tc.nc
    SQ = mybir.ActivationFunctionType.Square
    SQRT = mybir.ActivationFunctionType.Sqrt
    LN = mybir.ActivationFunctionType.Ln
    DR = mybir.MatmulPerfMode.DoubleRow

    consts = ctx.enter_context(tc.tile_pool(name="consts", bufs=1))
    stage = ctx.enter_context(tc.tile_pool(name="stage", bufs=3))
    eo = ctx.enter_context(tc.tile_pool(name="eo", bufs=3))
    magp = ctx.enter_context(tc.tile_pool(name="magp", bufs=2))
    sqp = ctx.enter_context(tc.tile_pool(name="sqp", bufs=2))
    outp = ctx.enter_context(tc.tile_pool(name="outp", bufs=2))
    ps_mm = ctx.enter_context(tc.tile_pool(name="ps_mm", bufs=4, space="PSUM"))
    ps_cq = ctx.enter_context(tc.tile_pool(name="ps_cq", bufs=2, space="PSUM"))

    # constants
    wc_sb = consts.tile([128, 8, 128], F16, tag="wc_sb")
    ws_sb = consts.tile([128, 8, 128], F16, tag="ws_sb")
    wsn_sb = consts.tile([128, 8, 128], F16, tag="wsn_sb")
    wc8_sb = [consts.tile([128, 8, 128], F8, tag=f"wc8_{i}",
                          name=f"wc8sb{i}") for i in range(1, NI)]
    ws8_sb = [consts.tile([128, 8, 128], F8, tag=f"ws8_{i}",
                          name=f"ws8sb{i}") for i in range(1, NI)]
    wq_sb = consts.tile([128, NI, NBINS], F16, tag="wq_sb")
    lnbias = consts.tile([128, 1], F32, tag="lnbias")
    nc.gpsimd.memset(lnbias[:], 1e-10)

    def emit_wc_load():
        # issued before any staging DMA: the very first matmul needs it
        nc.gpsimd.dma_start(wc_sb[:], wc.rearrange("a p f -> p a f"))

    def emit_weight_loads():
        # scalar queue carries only tables; gpsimd has wc, rev0, then these
        nc.scalar.dma_start(ws_sb[:], ws.rearrange("a p f -> p a f"))
        nc.gpsimd.dma_start(wsn_sb[:], wsn.rearrange("a p f -> p a f"))
        for i in range(1, NI):
            nc.gpsimd.dma_start(wc8_sb[i - 1][:],
                                wc8[i - 1].rearrange("a p f -> p a f"))
            nc.scalar.dma_start(ws8_sb[i - 1][:],
                                ws8[i - 1].rearrange("a p f -> p a f"))
        nc.scalar.dma_start(wq_sb[:], wq.rearrange("i p k -> p i k"))

    def emit_stage_dma(r, it, rev_q=None, split=False):
        """DMA of host-pretransposed chunks.

        split=True (tile 0 only): per-phase slices, so the first i0 matmul
        (which reads only phase a%4=0) starts after ~1/4 of the transfer.
        """
        dts = stage.tile([128, 4, Q], F16, tag="dts")
        rev = stage.tile([128, 4, Q], F16, tag="rev")
        if split:
            for ph in range(4):
                nc.sync.dma_start(dts[:, ph], xq[r, it, :, ph])
                (rev_q or nc.gpsimd).dma_start(rev[:, ph], zq[r, it, :, ph])
        else:
            nc.sync.dma_start(dts[:], xq[r, it])
            (rev_q or nc.gpsimd).dma_start(rev[:], zq[r, it])
        return dts, rev

    def emit_fold(it, dts, rev):
        """fp8 folded operands for freq blocks 1..3 (block 0 reads dts/rev
        directly, unfolded, in fp16).

        dts[p,ph,qq] = chunk(4qq+ph) sample p (shifted stream);
        rev[p,b,qq] = z-chunk(4qq+3-b), so partner of e8[:,a] (chunk
        4t+15-a) sits at rev[:, a%4, 3-(a//4)+t] -- two wide adds cover
        a=0..3 and a=4..7 with unit-stride operands.
        """
        T = T_SIZES[it]
        e8 = eo.tile([128, 8, T_ALLOC], F8, tag="e8")
        o8 = eo.tile([128, 8, T_ALLOC], F8, tag="o8")
        nc.vector.tensor_add(e8[:, 0:4, :T], dts[:, :, 0:T], rev[:, :, 3:3 + T])
        nc.vector.tensor_add(e8[:, 4:8, :T], dts[:, :, 1:1 + T], rev[:, :, 2:2 + T])
        nc.vector.tensor_sub(o8[:, 0:4, :T], dts[:, :, 0:T], rev[:, :, 3:3 + T])
        nc.vector.tensor_sub(o8[:, 4:8, :T], dts[:, :, 1:1 + T], rev[:, :, 2:2 + T])
        return dts, rev, e8, o8

    def emit_dft(r, it, dts, rev, e8, o8, split_tail=False):
        """DFT matmuls + magnitude for one frame tile (512 bins).

        Block 0 (fade-critical) is fp16 and unfolded: re = wc.x1 + wc.x2,
        im = ws.x1 - ws.x2 over the raw dts/rev streams (cos fold symmetry
        makes the rev-stream cos weights identical). Blocks 1..3 use fp8
        folded operands with DoubleRow (two 128-chunks per pass).
        """
        T = T_SIZES[it]
        H = T // 2
        mag = magp.tile([128, NI, T_ALLOC], F16, tag="mag")
        sq = sqp.tile([128, NI, T_ALLOC], F32, tag="sq")
        sq2 = sqp.tile([128, NI, T_ALLOC], F32, tag="sq2")
        d_ap = [dts[:, a % 4, a // 4: a // 4 + T] for a in range(8)]
        r_ap = [rev[:, a % 4, 3 - a // 4: 3 - a // 4 + T] for a in range(8)]
        for i in range(NI):
            ps_re = ps_mm.tile([128, T_ALLOC], F32, tag="mm")
            ps_im = ps_mm.tile([128, T_ALLOC], F32, tag="mm")
            if i == 0:
                for a in range(8):
                    nc.tensor.matmul(ps_re[:, :T], wc_sb[:, a], d_ap[a],
                                     start=(a == 0), stop=False)
                for a in range(8):
                    nc.tensor.matmul(ps_re[:, :T], wc_sb[:, a], r_ap[a],
                                     start=False, stop=(a == 7))
                for a in range(8):
                    nc.tensor.matmul(ps_im[:, :T], ws_sb[:, a], d_ap[a],
                                     start=(a == 0), stop=False)
                for a in range(8):
                    nc.tensor.matmul(ps_im[:, :T], wsn_sb[:, a], r_ap[a],
                                     start=False, stop=(a == 7))
            else:
                # full-T DoubleRow: moving rows = 2T = 856, but the pair
                # consumption halves the effective pass count so the PSUM
                # output span (T <= 512, one bank) is the binding limit
                for ps, wt, op in ((ps_re, wc8_sb[i - 1], e8),
                                   (ps_im, ws8_sb[i - 1], o8)):
                    for a in range(0, 8, 2):
                        nc.tensor.matmul(
                            ps[:, :T], wt[:, a:a + 2, :],
                            op[:, a:a + 2, :T],
                            start=(a == 0), stop=(a == 6),
                            perf_mode=DR,
                        )
            nc.scalar.activation(sq[:, i, :T], ps_re[:, :T], SQ)
            nc.scalar.activation(sq2[:, i, :T], ps_im[:, :T], SQ)
        # one wide |.|^2 sum + sqrt for the whole tile; the add lives on
        # gpsimd so the DVE queue carries nothing but the fold stream
        if not split_tail:
            # one wide |.|^2 sum + sqrt for the whole tile; the add lives on
            # gpsimd so the DVE queue carries nothing but the fold stream
            nc.gpsimd.tensor_add(sq[:, :, :T], sq[:, :, :T], sq2[:, :, :T])
            nc.scalar.activation(mag[:, :, :T], sq[:, :, :T], SQRT)
        else:
            # last tile: per-block sum/sqrt on the now-idle DVE so the final
            # cqt matmuls can fire incrementally as each mag block lands
            for i in range(NI):
                nc.vector.tensor_add(sq[:, i, :T], sq[:, i, :T], sq2[:, i, :T])
                nc.scalar.activation(mag[:, i, :T], sq[:, i, :T], SQRT)
        return mag

    def emit_cqt(r, it, mag, c0=0, c1=None, tail=False):
        """CQT projection, log10, store (over frame columns [c0, c1))."""
        T = T_SIZES[it]
        f0 = T_STARTS[it]
        if c1 is None:
            c1 = T
        V = min(c1, NF - f0)         # valid (non-garbage) frames
        ps_c = ps_cq.tile([NBINS, T_ALLOC], F32, tag="ps_c")
        for i in range(NI):
            nc.tensor.matmul(
                ps_c[:, c0:c1], wq_sb[:, i, :], mag[:, i, c0:c1],
                start=(i == 0), stop=(i == NI - 1),
            )
        outt = outp.tile([NBINS, T_ALLOC], F32, tag="outt")
        nc.scalar.activation(outt[:, c0:V], ps_c[:, c0:V], LN,
                             bias=lnbias[:NBINS])
        if tail:
            # epilogue: DVE is idle and this avoids Ln<->Copy act-table thrash
            nc.vector.tensor_scalar_mul(outt[:, c0:V], outt[:, c0:V], LOG10E)
        else:
            nc.scalar.mul(outt[:, c0:V], outt[:, c0:V], LOG10E)
        nc.sync.dma_start(out[r, :, f0 + c0: f0 + V], outt[:, c0:V])

    # software pipeline: DMA prefetches two tiles ahead, folds run just-in-
    # time after the current tile's DFT so the in-order DVE queue never
    # blocks the magnitude chain, and cqt(k-1) is emitted AFTER dft(k) so
    # the PE never waits on the k-1 magnitude chain.
    # Slot k: [dma k+2][dft k][cqt k-1][fold k+1]
    tiles = [(r, it) for r in range(ROWS_PER_CORE) for it in range(6)]
    # wc first on gpsimd, then tile-0 staging (phase-sliced), then the rest
    # of the tables -- every queue feeds its first consumer in need-order
    emit_wc_load()
    dmas = {0: emit_stage_dma(*tiles[0], split=True)}
    emit_weight_loads()
    dmas[1] = emit_stage_dma(*tiles[1])
    folded = {0: emit_fold(tiles[0][1], *dmas.pop(0))}
    pending = None          # (r, it, mag) awaiting cqt
    last = len(tiles) - 1
    for k, (r, it) in enumerate(tiles):
        if k + 2 < len(tiles):
            dmas[k + 2] = emit_stage_dma(*tiles[k + 2])
        mag = emit_dft(r, it, *folded.pop(k), split_tail=(k == last))
        if pending is not None:
            emit_cqt(*pending)
        pending = (r, it, mag)
        if k + 1 < len(tiles):
            folded[k + 1] = emit_fold(tiles[k + 1][1], *dmas.pop(k + 1))
    emit_cqt(*pending)


_PROGRAM_CACHE = {}


def _get_program():
    if "nc" not in _PROGRAM_CACHE:
        _PROGRAM_CACHE["nc"] = _build_program()
    return _PROGRAM_CACHE["nc"]


def kernel(audio):
    audio = np.asarray(audio, dtype=np.float32)
    assert audio.shape == (B, L), audio.shape

    # host data movement: reflect pad + fp16, then per-tile pretransposed
    # blocks in the exact SBUF layout the device consumes (pure gathers).
    xpad = np.zeros((B, PADLEN), dtype=np.float16)
    xpad[:, :LP] = np.pad(audio, ((0, 0), (PAD, PAD)), mode="reflect")

    xqh = np.empty((B, 6, 128, 4, Q), dtype=np.float16)
    zqh = np.empty((B, 6, 128, 4, Q), dtype=np.float16)
    for it in range(6):
        base = 128 * 4 * T_STARTS[it]
        # xq[p, ph, q] = xpad[base + 128*(4q+ph) + p + 1]
        seg = xpad[:, base + 1: base + 1 + WCH * 128].reshape(B, Q, 4, 128)
        xqh[:, it] = seg.transpose(0, 3, 2, 1)
        # zq[p, b, q] = xpad[base + 128*(4q+3-b) + 127 - p]
        zseg = xpad[:, base: base + WCH * 128].reshape(B, Q, 4, 128)
        zqh[:, it] = zseg[:, :, ::-1, ::-1].transpose(0, 3, 2, 1)

    import ml_dtypes
    wc, ws, wq = _host_tables()
    # block 0 fp16 [a, p, f]; blocks 1..3 fp8 [i-1, a, p, f]
    wc0 = np.ascontiguousarray(wc[:, :128].reshape(8, 128, 128))
    ws0 = np.ascontiguousarray(ws[:, :128].reshape(8, 128, 128))
    wsn0 = np.ascontiguousarray(-ws0)
    e4m3 = ml_dtypes.float8_e4m3fn
    wc8 = np.ascontiguousarray(wc[:, 128:].reshape(8, 128, NI - 1, 128)
                               .transpose(2, 0, 1, 3)).astype(e4m3)
    ws8 = np.ascontiguousarray(ws[:, 128:].reshape(8, 128, NI - 1, 128)
                               .transpose(2, 0, 1, 3)).astype(e4m3)
    wq = np.ascontiguousarray(wq.reshape(NI, 128, NBINS))
    nc = _get_program()

    in_maps = []
    for c in range(NCORES):
        rows = slice(ROWS_PER_CORE * c, ROWS_PER_CORE * (c + 1))
        in_maps.append({
            "xq": np.ascontiguousarray(xqh[rows]),
            "zq": np.ascontiguousarray(zqh[rows]),
            "wc": wc0, "ws": ws0, "wsn": wsn0,
            "wc8": wc8, "ws8": ws8, "wq": wq,
        })

    res = run_bass_kernel_spmd(nc, in_maps, core_ids=list(range(NCORES)))
    out = np.concatenate([res.results[c]["out"] for c in range(NCORES)], axis=0)
    return np.ascontiguousarray(out, dtype=np.float32)


# revision 62
# speedup vs baseline: 1.0944x; 1.0898x over previous
"""CQT extractor kernel for Trainium2 (8 NeuronCores, data-parallel over batch).

Pipeline per core (2 audio rows): STFT-as-matmul truncated to the lowest 512
of 1025 rfft bins (CQT weights above ~5.5 kHz are < 2e-3 and contribute ~1e-4
relative error), hybrid precision:
  - freq block 0 (fade-critical narrow CQT bins): fp16, unfolded -- both
    staged streams feed the PE directly (cos fold symmetry reuses one table;
    a negated-sin table handles the rev stream for im).
  - freq blocks 1..3 (wide, noise-averaging bins): fp8 e4m3 with Hermitian
    folding and DoubleRow matmuls (two 128-chunk k-tiles per pass).
Magnitude via ACT Square/Sqrt with the |.|^2 sum on gpsimd, CQT projection
matmul in fp16, log10 via Ln.

The host pre-transposes the reflect-padded audio into per-tile SBUF-layout
blocks (pure data movement), so the device does no transposes or PSUM->SBUF
staging copies; the in-order engine queues carry disjoint streams (DVE: fp8
folds, Pool: magnitude sums, scalar: activations) with DMA prefetched two
tiles ahead.
"""

import math
from contextlib import ExitStack

import numpy as np

import concourse.tile as tile
from concourse import bacc, mybir
from concourse.bass_utils import run_bass_kernel_spmd

# ---- problem constants (hardcoded per contest rules) ----
B = 16
L = 1310720
SR = 22050
HOP = 512
NFFT = 2048
NBINS = 84
BPO = 12
FMIN = 27.5

NF = 1 + L // HOP            # 2561 frames
PAD = NFFT // 2              # 1024
LP = L + 2 * PAD             # 1312768 reflect-padded length

NCORES = 8
ROWS_PER_CORE = B // NCORES  # 2

NFREQ = 512                  # truncated rfft bins (of 1025)
NI = NFREQ // 128            # 4 freq blocks

# frame tiling: 6 uniform tiles of 428 frames; frames past NF-1 are computed
# on zero padding and never written out
T_SIZES = [428] * 6
T_STARTS = [428 * i for i in range(6)]
T_ALLOC = 428

NGRP = 14                    # 128-chunk transpose groups per frame tile
WCH = NGRP * 128             # 1792 chunks staged per frame tile
Q = WCH // 4                 # 448 per-phase chunk columns
NCH_PAD = 4 * T_STARTS[-1] + WCH + 1   # chunks incl. zero pad (+1 for +1 shift)
PADLEN = 128 * NCH_PAD

F32 = mybir.dt.float32
F16 = mybir.dt.float16
F8 = mybir.dt.float8e4
LOG10E = 1.0 / math.log(10.0)


def _host_tables():
    """Folded DFT matrices (512 bins) and CQT weights, float64 -> float16."""
    j = np.arange(1024)
    n = (j + 1).astype(np.float64)          # contraction index j <-> sample n=j+1
    win = 0.5 * (1.0 - np.cos(2.0 * np.pi * n / NFFT))
    ang = 2.0 * np.pi * np.outer(n, np.arange(NFREQ, dtype=np.float64)) / NFFT
    wc = win[:, None] * np.cos(ang)
    ws = win[:, None] * np.sin(ang)
    wc[1023] *= 0.5           # n=1024 term is double-counted by the fold
    ws[1023] = 0.0
    sf = np.fft.rfftfreq(NFFT, 1.0 / SR)[:NFREQ]
    cf = FMIN * 2.0 ** (np.arange(NBINS, dtype=np.float64) / BPO)
    wq = np.exp(-np.abs(sf[:, None] - cf[None, :]) / (0.1 * cf[None, :]))
    return (
        np.ascontiguousarray(wc, dtype=np.float16),
        np.ascontiguousarray(ws, dtype=np.float16),
        np.ascontiguousarray(wq, dtype=np.float16),
    )


def _build_program():
    nc = bacc.Bacc("TRN2", target_bir_lowering=False, debug=False,
                   num_devices=NCORES)
    # host-pretransposed staged audio, per frame tile:
    # xq[r, it, p, ph, q] = xpad[128*(4*T_STARTS[it] + 4q + ph) + p + 1]
    # zq[r, it, p, b,  q] = xpad[128*(4*T_STARTS[it] + 4q + 3 - b) + 127 - p]
    xq = nc.dram_tensor("xq", [ROWS_PER_CORE, 6, 128, 4, Q], F16,
                        kind="ExternalInput").ap()
    zq = nc.dram_tensor("zq", [ROWS_PER_CORE, 6, 128, 4, Q], F16,
                        kind="ExternalInput").ap()
    # fp16 tables for freq block 0 (fade-critical), incl. negated sin for the
    # unfolded rev-stream im accumulation; fp8 tables for blocks 1..3
    wc = nc.dram_tensor("wc", [8, 128, 128], F16, kind="ExternalInput").ap()
    ws = nc.dram_tensor("ws", [8, 128, 128], F16, kind="ExternalInput").ap()
    wsn = nc.dram_tensor("wsn", [8, 128, 128], F16, kind="ExternalInput").ap()
    wc8 = nc.dram_tensor("wc8", [NI - 1, 8, 128, 128], F8,
                         kind="ExternalInput").ap()
    ws8 = nc.dram_tensor("ws8", [NI - 1, 8, 128, 128], F8,
                         kind="ExternalInput").ap()
    wq = nc.dram_tensor("wq", [NI, 128, NBINS], F16, kind="ExternalInput").ap()
    out = nc.dram_tensor("out", [ROWS_PER_CORE, NBINS, NF], F32,
                         kind="ExternalOutput").ap()

    with tile.TileContext(nc) as tc:
        with ExitStack() as ctx:
            _emit(ctx, tc, xq, zq, wc, ws, wsn, wc8, ws8, wq, out)
    nc.compile()
    return nc


def _emit(ctx, tc, xq, zq, wc, ws, wsn, wc8, ws8, wq, out):
    nc = tc.nc
    SQ = mybir.ActivationFunctionType.Square
    SQRT = mybir.ActivationFunctionType.Sqrt
    LN = mybir.ActivationFunctionType.Ln
    DR = mybir.MatmulPerfMode.DoubleRow

    consts = ctx.enter_context(tc.tile_pool(name="consts", bufs=1))
    stage = ctx.enter_context(tc.tile_pool(name="stage", bufs=3))
    eo = ctx.enter_context(tc.tile_pool(name="eo", bufs=3))
    magp = ctx.enter_context(tc.tile_pool(name="magp", bufs=2))
    sqp = ctx.enter_context(tc.tile_pool(name="sqp", bufs=2))
    outp = ctx.enter_context(tc.tile_pool(name="outp", bufs=2))
    ps_mm = ctx.enter_context(tc.tile_pool(name="ps_mm", bufs=4, space="PSUM"))
    ps_cq = ctx.enter_context(tc.tile_pool(name="ps_cq", bufs=2, space="PSUM"))

    # constants
    wc_sb = consts.tile([128, 8, 128], F16, tag="wc_sb")
    ws_sb = consts.tile([128, 8, 128], F16, tag="ws_sb")
    wsn_sb = consts.tile([128, 8, 128], F16, tag="wsn_sb")
    wc8_sb = [consts.tile([128, 8, 128], F8, tag=f"wc8_{i}",
                          name=f"wc8sb{i}") for i in range(1, NI)]
    ws8_sb = [consts.tile([128, 8, 128], F8, tag=f"ws8_{i}",
                          name=f"ws8sb{i}") for i in range(1, NI)]
    wq_sb = consts.tile([128, NI, NBINS], F16, tag="wq_sb")
    lnbias = consts.tile([128, 1], F32, tag="lnbias")
    nc.gpsimd.memset(lnbias[:], 1e-10)

    def emit_wc_load():
        # issued before any staging DMA: the very first matmul needs it
        nc.gpsimd.dma_start(wc_sb[:], wc.rearrange("a p f -> p a f"))

    def emit_weight_loads():
        # scalar queue carries only tables; gpsimd has wc, rev0, then these
        nc.scalar.dma_start(ws_sb[:], ws.rearrange("a p f -> p a f"))
        nc.gpsimd.dma_start(wsn_sb[:], wsn.rearrange("a p f -> p a f"))
        for i in range(1, NI):
            nc.gpsimd.dma_start(wc8_sb[i - 1][:],
                                wc8[i - 1].rearrange("a p f -> p a f"))
            nc.scalar.dma_start(ws8_sb[i - 1][:],
                                ws8[i - 1].rearrange("a p f -> p a f"))
        nc.scalar.dma_start(wq_sb[:], wq.rearrange("i p k -> p i k"))

    def emit_stage_dma(r, it, rev_q=None, split=False):
        """DMA of host-pretransposed chunks.

        split=True (tile 0 only): per-phase slices, so the first i0 matmul
        (which reads only phase a%4=0) starts after ~1/4 of the transfer.
        """
        dts = stage.tile([128, 4, Q], F16, tag="dts")
        rev = stage.tile([128, 4, Q], F16, tag="rev")
        if split:
            for ph in range(4):
                nc.sync.dma_start(dts[:, ph], xq[r, it, :, ph])
                (rev_q or nc.gpsimd).dma_start(rev[:, ph], zq[r, it, :, ph])
        else:
            nc.sync.dma_start(dts[:], xq[r, it])
            (rev_q or nc.gpsimd).dma_start(rev[:], zq[r, it])
        return dts, rev

    def emit_fold(it, dts, rev, startup=False):
        """fp8 folded operands for freq blocks 1..3 (block 0 reads dts/rev
        directly, unfolded, in fp16).

        dts[p,ph,qq] = chunk(4qq+ph) sample p (shifted stream);
        rev[p,b,qq] = z-chunk(4qq+3-b), so partner of e8[:,a] (chunk
        4t+15-a) sits at rev[:, a%4, 3-(a//4)+t] -- two wide adds cover
        a=0..3 and a=4..7 with unit-stride operands.
        """
        T = T_SIZES[it]
        e8 = eo.tile([128, 8, T_ALLOC], F8, tag="e8")
        o8 = eo.tile([128, 8, T_ALLOC], F8, tag="o8")
        if startup:
            # tile 0 only: second halves ride the (still idle) Pool so the
            # DVE fold stream finishes early and runs a tile ahead of the PE
            # from the very first slot instead of converging over 3 slots
            nc.vector.tensor_add(e8[:, 0:4, :T], dts[:, :, 0:T],
                                 rev[:, :, 3:3 + T])
            nc.vector.tensor_sub(o8[:, 0:4, :T], dts[:, :, 0:T],
                                 rev[:, :, 3:3 + T])
            nc.gpsimd.tensor_add(e8[:, 4:8, :T], dts[:, :, 1:1 + T],
                                 rev[:, :, 2:2 + T])
            nc.gpsimd.tensor_sub(o8[:, 4:8, :T], dts[:, :, 1:1 + T],
                                 rev[:, :, 2:2 + T])
        else:
            nc.vector.tensor_add(e8[:, 0:4, :T], dts[:, :, 0:T],
                                 rev[:, :, 3:3 + T])
            nc.vector.tensor_add(e8[:, 4:8, :T], dts[:, :, 1:1 + T],
                                 rev[:, :, 2:2 + T])
            nc.vector.tensor_sub(o8[:, 0:4, :T], dts[:, :, 0:T],
                                 rev[:, :, 3:3 + T])
            nc.vector.tensor_sub(o8[:, 4:8, :T], dts[:, :, 1:1 + T],
                                 rev[:, :, 2:2 + T])
        return dts, rev, e8, o8

    def emit_dft(r, it, dts, rev, e8, o8, split_tail=False):
        """DFT matmuls + magnitude for one frame tile (512 bins).

        Block 0 (fade-critical) is fp16 and unfolded: re = wc.x1 + wc.x2,
        im = ws.x1 - ws.x2 over the raw dts/rev streams (cos fold symmetry
        makes the rev-stream cos weights identical). Blocks 1..3 use fp8
        folded operands with DoubleRow (two 128-chunks per pass).
        """
        T = T_SIZES[it]
        H = T // 2
        mag = magp.tile([128, NI, T_ALLOC], F16, tag="mag")
        sq = sqp.tile([128, NI, T_ALLOC], F32, tag="sq")
        sq2 = sqp.tile([128, NI, T_ALLOC], F32, tag="sq2")
        d_ap = [dts[:, a % 4, a // 4: a // 4 + T] for a in range(8)]
        r_ap = [rev[:, a % 4, 3 - a // 4: 3 - a // 4 + T] for a in range(8)]
        for i in range(NI):
            ps_re = ps_mm.tile([128, T_ALLOC], F32, tag="mm")
            ps_im = ps_mm.tile([128, T_ALLOC], F32, tag="mm")
            if i == 0:
                for a in range(8):
                    nc.tensor.matmul(ps_re[:, :T], wc_sb[:, a], d_ap[a],
                                     start=(a == 0), stop=False)
                for a in range(8):
                    nc.tensor.matmul(ps_re[:, :T], wc_sb[:, a], r_ap[a],
                                     start=False, stop=(a == 7))
                for a in range(8):
                    nc.tensor.matmul(ps_im[:, :T], ws_sb[:, a], d_ap[a],
                                     start=(a == 0), stop=False)
                for a in range(8):
                    nc.tensor.matmul(ps_im[:, :T], wsn_sb[:, a], r_ap[a],
                                     start=False, stop=(a == 7))
            else:
                # full-T DoubleRow: moving rows = 2T = 856, but the pair
                # consumption halves the effective pass count so the PSUM
                # output span (T <= 512, one bank) is the binding limit
                for ps, wt, op in ((ps_re, wc8_sb[i - 1], e8),
                                   (ps_im, ws8_sb[i - 1], o8)):
                    for a in range(0, 8, 2):
                        nc.tensor.matmul(
                            ps[:, :T], wt[:, a:a + 2, :],
                            op[:, a:a + 2, :T],
                            start=(a == 0), stop=(a == 6),
                            perf_mode=DR,
                        )
            nc.scalar.activation(sq[:, i, :T], ps_re[:, :T], SQ)
            nc.scalar.activation(sq2[:, i, :T], ps_im[:, :T], SQ)
        # one wide |.|^2 sum + sqrt for the whole tile; the add lives on
        # gpsimd so the DVE queue carries nothing but the fold stream
        if not split_tail:
            # one wide |.|^2 sum + sqrt for the whole tile; the add lives on
            # gpsimd so the DVE queue carries nothing but the fold stream
            nc.gpsimd.tensor_add(sq[:, :, :T], sq[:, :, :T], sq2[:, :, :T])
            nc.scalar.activation(mag[:, :, :T], sq[:, :, :T], SQRT)
        else:
            # last tile: per-block sum/sqrt on the now-idle DVE so the final
            # cqt matmuls can fire incrementally as each mag block lands
            for i in range(NI):
                nc.vector.tensor_add(sq[:, i, :T], sq[:, i, :T], sq2[:, i, :T])
                nc.scalar.activation(mag[:, i, :T], sq[:, i, :T], SQRT)
        return mag

    def emit_cqt(r, it, mag, c0=0, c1=None, tail=False):
        """CQT projection, log10, store (over frame columns [c0, c1))."""
        T = T_SIZES[it]
        f0 = T_STARTS[it]
        if c1 is None:
            c1 = T
        V = min(c1, NF - f0)         # valid (non-garbage) frames
        ps_c = ps_cq.tile([NBINS, T_ALLOC], F32, tag="ps_c")
        for i in range(NI):
            nc.tensor.matmul(
                ps_c[:, c0:c1], wq_sb[:, i, :], mag[:, i, c0:c1],
                start=(i == 0), stop=(i == NI - 1),
            )
        outt = outp.tile([NBINS, T_ALLOC], F32, tag="outt")
        nc.scalar.activation(outt[:, c0:V], ps_c[:, c0:V], LN,
                             bias=lnbias[:NBINS])
        if tail:
            # epilogue: DVE is idle and this avoids Ln<->Copy act-table thrash
            nc.vector.tensor_scalar_mul(outt[:, c0:V], outt[:, c0:V], LOG10E)
        else:
            nc.scalar.mul(outt[:, c0:V], outt[:, c0:V], LOG10E)
        nc.sync.dma_start(out[r, :, f0 + c0: f0 + V], outt[:, c0:V])

    # software pipeline: DMA prefetches two tiles ahead, folds run just-in-
    # time after the current tile's DFT so the in-order DVE queue never
    # blocks the magnitude chain, and cqt(k-1) is emitted AFTER dft(k) so
    # the PE never waits on the k-1 magnitude chain.
    # Slot k: [dma k+2][dft k][cqt k-1][fold k+1]
    tiles = [(r, it) for r in range(ROWS_PER_CORE) for it in range(6)]
    # wc first on gpsimd, then tile-0 staging (phase-sliced), then the rest
    # of the tables -- every queue feeds its first consumer in need-order
    emit_wc_load()
    dmas = {0: emit_stage_dma(*tiles[0], split=True)}
    emit_weight_loads()
    dmas[1] = emit_stage_dma(*tiles[1])
    folded = {0: emit_fold(tiles[0][1], *dmas.pop(0), startup=True)}
    pending = None          # (r, it, mag) awaiting cqt
    last = len(tiles) - 1
    for k, (r, it) in enumerate(tiles):
        if k + 2 < len(tiles):
            dmas[k + 2] = emit_stage_dma(*tiles[k + 2])
        mag = emit_dft(r, it, *folded.pop(k), split_tail=(k == last))
        if pending is not None:
            emit_cqt(*pending)
        pending = (r, it, mag)
        if k + 1 < len(tiles):
            folded[k + 1] = emit_fold(tiles[k + 1][1], *dmas.pop(k + 1))
    emit_cqt(*pending)


_PROGRAM_CACHE = {}


def _get_program():
    if "nc" not in _PROGRAM_CACHE:
        _PROGRAM_CACHE["nc"] = _build_program()
    return _PROGRAM_CACHE["nc"]


def kernel(audio):
    audio = np.asarray(audio, dtype=np.float32)
    assert audio.shape == (B, L), audio.shape

    # host data movement: reflect pad + fp16, then per-tile pretransposed
    # blocks in the exact SBUF layout the device consumes (pure gathers).
    xpad = np.zeros((B, PADLEN), dtype=np.float16)
    xpad[:, :LP] = np.pad(audio, ((0, 0), (PAD, PAD)), mode="reflect")

    xqh = np.empty((B, 6, 128, 4, Q), dtype=np.float16)
    zqh = np.empty((B, 6, 128, 4, Q), dtype=np.float16)
    for it in range(6):
        base = 128 * 4 * T_STARTS[it]
        # xq[p, ph, q] = xpad[base + 128*(4q+ph) + p + 1]
        seg = xpad[:, base + 1: base + 1 + WCH * 128].reshape(B, Q, 4, 128)
        xqh[:, it] = seg.transpose(0, 3, 2, 1)
        # zq[p, b, q] = xpad[base + 128*(4q+3-b) + 127 - p]
        zseg = xpad[:, base: base + WCH * 128].reshape(B, Q, 4, 128)
        zqh[:, it] = zseg[:, :, ::-1, ::-1].transpose(0, 3, 2, 1)

    import ml_dtypes
    wc, ws, wq = _host_tables()
    # block 0 fp16 [a, p, f]; blocks 1..3 fp8 [i-1, a, p, f]
    wc0 = np.ascontiguousarray(wc[:, :128].reshape(8, 128, 128))
    ws0 = np.ascontiguousarray(ws[:, :128].reshape(8, 128, 128))
    wsn0 = np.ascontiguousarray(-ws0)
    e4m3 = ml_dtypes.float8_e4m3fn
    wc8 = np.ascontiguousarray(wc[:, 128:].reshape(8, 128, NI - 1, 128)
                               .transpose(2, 0, 1, 3)).astype(e4m3)
    ws8 = np.ascontiguousarray(ws[:, 128:].reshape(8, 128, NI - 1, 128)
                               .transpose(2, 0, 1, 3)).astype(e4m3)
    wq = np.ascontiguousarray(wq.reshape(NI, 128, NBINS))
    nc = _get_program()

    in_maps = []
    for c in range(NCORES):
        rows = slice(ROWS_PER_CORE * c, ROWS_PER_CORE * (c + 1))
        in_maps.append({
            "xq": np.ascontiguousarray(xqh[rows]),
            "zq": np.ascontiguousarray(zqh[rows]),
            "wc": wc0, "ws": ws0, "wsn": wsn0,
            "wc8": wc8, "ws8": ws8, "wq": wq,
        })

    res = run_bass_kernel_spmd(nc, in_maps, core_ids=list(range(NCORES)))
    out = np.concatenate([res.results[c]["out"] for c in range(NCORES)], axis=0)
    return np.ascontiguousarray(out, dtype=np.float32)
